# revision 6
# baseline (speedup 1.0000x reference)
"""AttentionGate kernel for Trainium2 (8 NeuronCores, pure data parallel).

Reference computation (per pixel p, channels c):
    t[p] = sum_c input_[p,c]*wt[c] + bt
    g[p] = sum_c gating [p,c]*wg[c] + bg
    x[p] = sigmoid(w2 * relu(t[p]+g[p]) + b2)
    out[p,c] = input_[p,c] * x[p]

HBM-bandwidth-bound kernel (~358 GB/s per NeuronCore).  All HBM I/O is
bf16: the host rounds both inputs to bf16 (uniform 2^-9 relative error --
unlike fp16 there is no subnormal flush for tiny inputs) and additionally
interleaves them per pixel row into one [ROWS, 512] tensor (x_row ||
g_row), so each pixel row is a single contiguous 512-element run in both
HBM and SBUF.  The kernel computes the dots with f32 accumulation and
stores a bf16 output the host up-converts exactly to f32.  48 MB moves
per core instead of the f32 baseline's 96 MB.

Layout: partition p owns 256 consecutive pixel rows; a block of sz rows
loads as one DMA with sz KB contiguous per partition.  Per pixel row one
fused DVE scalar_tensor_tensor computes the product against [wt;wg] AND
its free-dim sum (512-wide dot, f32 accumulator).  ScalarE applies
relu(+bt+bg) and sigmoid(w2*x+b2) batched per block.  The per-pixel gate
multiply is split between ScalarE (activation Copy with per-partition
scale) and DVE (tensor_scalar) to balance the two engines' busy time.
Stores issue from the ACT HWDGE ring, loads from the SP ring.

Sharding: batch dim 16 -> 2 batches per core, weights replicated.
"""

import sys

import numpy as np

for _p in ("/opt/trn_rl_repo", "/opt/trn_rl_repo/concourse"):
    if _p not in sys.path:
        sys.path.append(_p)

B, H, W, C = 16, 128, 128, 256
NCORES = 8
ROWS = (B // NCORES) * H * W          # pixels per core = 32768
P = 128                                # partitions
CAT = 2 * C                            # input || gating channels
RPP = ROWS // P                        # pixel rows owned per partition = 256
LB = 32                                # steady-state rows per load block
BLOCK_SIZES = [4, 4, 8, 16] + [LB] * 6 + [16, 8, 4, 4]
assert sum(BLOCK_SIZES) == RPP
# Gate-multiplies of the last TAIL_DVE_ROWS pixel rows run on DVE
# (tensor_scalar, 285ns) instead of ACT (Copy w/ scale, ~500ns): by then
# DVE has finished its dots and would otherwise idle while ACT drains.
TAIL_DVE_ROWS = 40
# Mid-stream, ACT takes over the dot-reduce of ACT_RED_PER_BLOCK rows per
# full block (DVE does a plain tensor_tensor product, ACT a Copy+accum):
# shifts ~180ns/row off the DVE critical path onto ACT's slack.
ACT_RED_PER_BLOCK = 3

_PATCHED = False


def _apply_compat_patches():
    """Work around two ISA-encoding gaps in this container's neuronxcc walrus:

    1. EVENT_SEMAPHORE_RANGE_CLEAR (emitted by the TileContext teardown's
       sem_clear) fails codegen with "ISA wrong length".  Re-execution is
       safe without it (verified on HW), so skip the clear.
    2. The teardown drain carries one sem-wait per logical processor; this
       walrus rejects >1 sync-wait command on a NO_STRUCT ctrl instruction
       ("Too many sync wait commands").  Split the final clock wait into one
       NOP per processor instead.
    """
    global _PATCHED
    if _PATCHED:
        return
    _PATCHED = True

    import concourse.bass as bass
    import concourse.tile as tile_mod
    from bass_rust import ScopedClock, VectorClock
    from concourse.bass import SemaphoreHandle, compact_to_ranges

    def patched_clear(self, sems):
        if not sems:
            return
        sem_nums = [s.num if isinstance(s, SemaphoreHandle) else s for s in sems]
        for sem_range in compact_to_ranges(sem_nums):
            assert self._state.free_isdisjoint(sem_range)
            self.gpsimd.dma_reset(sem_range)
        self._state.prepend_free_semaphores(sem_nums)
        for poison_set in self._tile_sem_poison_stack:
            poison_set.update(sem_nums)

    bass.Bass.clear_and_free_semaphores = patched_clear

    def patched_drain_and_barrier(self, tick_clock, wait_clock):
        gc = tick_clock.global_clock
        for p in range(len(gc)):
            if gc[p] <= 0:
                continue
            vc = VectorClock()
            vc.require_at_least(p, gc[p])
            di = self.nc.sync.nop(nofuse=True)
            wait_clock.add_sem_waits(di.ins, ScopedClock({None: vc}))
        assert self.sems is not None
        popped = self.nc._tile_sem_poison_stack.pop()
        assert popped is self._sem_poison
        # bookkeeping only: recycle sem ids; no dma_reset (the body issues
        # no SWDGE DMAs) and no second barrier -> shorter kernel tail
        sems = list(self.sems.allocated().values())
        from concourse.bass import SemaphoreHandle
        sem_nums = [s.num if isinstance(s, SemaphoreHandle) else s for s in sems]
        self.nc._state.prepend_free_semaphores(sem_nums)
        for poison_set in self.nc._tile_sem_poison_stack:
            poison_set.update(sem_nums)

    tile_mod.TileContext._drain_and_barrier = patched_drain_and_barrier


def _split_multi_waits(nc):
    """This walrus build only encodes ONE sync-wait command per TPB
    instruction.  Hoist all-but-the-last wait of any instruction onto
    freshly inserted same-engine NoOps placed directly before it."""
    import concourse.mybir as mybir

    for f in nc.m.functions:
        for bb in f.blocks:
            insts = bb.instructions  # live list
            i = 0
            while i < len(insts):
                inst = insts[i]
                si = getattr(inst, "sync_info", None)
                if si is not None and len(si.on_wait) > 1:
                    extra, last = list(si.on_wait[:-1]), si.on_wait[-1]
                    for w in extra:
                        nop = mybir.InstNoOp(
                            name=nc.get_next_instruction_name(),
                            engine=inst.engine,
                            sync_info=mybir.SyncInfo(on_wait=[w], on_update=[]),
                            bass_nofuse=True,
                        )
                        insts.insert(i, nop)
                        i += 1
                    inst.sync_info = mybir.SyncInfo(
                        on_wait=[last], on_update=list(si.on_update)
                    )
                i += 1


def _build_program(bt, bg, w2, b2):
    import concourse.bass as bass
    import concourse.mybir as mybir
    from concourse.tile import TileContext

    nc = bass.Bass()
    bf16 = mybir.dt.bfloat16
    f32 = mybir.dt.float32
    xg_d = nc.declare_dram_parameter("xg", [ROWS, CAT], bf16, isOutput=False)
    w_d = nc.declare_dram_parameter("wcat", [P, CAT], bf16, isOutput=False)
    o_d = nc.declare_dram_parameter("out", [ROWS, C], bf16, isOutput=True)

    xg_v = xg_d[:].rearrange("(p q) c -> p q c", p=P)
    o_v = o_d[:].rearrange("(p q) c -> p q c", p=P)

    with TileContext(nc) as tc:
        with (
            tc.tile_pool(name="wp", bufs=1) as wp,
            tc.tile_pool(name="io", bufs=3) as io,
            tc.tile_pool(name="op", bufs=3) as op,
            tc.tile_pool(name="sc", bufs=2) as sc,
            tc.tile_pool(name="sm", bufs=4) as sm,
        ):
            wcat = wp.tile([P, CAT], bf16)   # wt || wg
            nc.sync.dma_start(wcat[:], w_d[:])
            b2t = wp.tile([P, 1], f32)
            nc.vector.memset(b2t[:], float(b2))
            bias_t = wp.tile([P, 1], f32)
            nc.vector.memset(bias_t[:], float(bt + bg))

            off = 0
            for sz in BLOCK_SIZES:
                span = slice(off, off + sz)
                is_full = sz == LB
                xg = io.tile([P, LB, CAT], bf16, tag="xg")
                nc.sync.dma_start(xg[:, 0:sz, :], xg_v[:, span, :])
                ob = op.tile([P, LB, C], bf16, tag="ob")
                s_blk = sm.tile([P, LB], f32, tag="s")
                n_red = ACT_RED_PER_BLOCK if is_full else 0
                for r in range(sz):
                    dump = sc.tile([P, CAT], bf16, tag="dump")
                    if r < n_red:
                        # products on DVE (plain TT, 2x mode), reduce on ACT
                        nc.vector.tensor_tensor(
                            dump[:], xg[:, r, :], wcat[:],
                            mybir.AluOpType.mult,
                        )
                        red = sc.tile([P, CAT], bf16, tag="red")
                        nc.scalar.activation(
                            red[:], dump[:],
                            mybir.ActivationFunctionType.Copy,
                            accum_out=s_blk[:, r : r + 1],
                        )
                    else:
                        # dump = xg_row * [wt;wg]; accum = 512-dot = t+g
                        nc.vector.scalar_tensor_tensor(
                            out=dump[:],
                            in0=xg[:, r, :],
                            scalar=0.0,
                            in1=wcat[:],
                            op0=mybir.AluOpType.bypass,
                            op1=mybir.AluOpType.mult,
                            accum_out=s_blk[:, r : r + 1],
                        )
                xs_blk = sm.tile([P, LB], f32, tag="xs")
                nc.scalar.activation(
                    xs_blk[:, 0:sz], s_blk[:, 0:sz],
                    mybir.ActivationFunctionType.Relu,
                    bias=bias_t[:],
                )
                xsig_blk = sm.tile([P, LB], f32, tag="xsig")
                nc.scalar.activation(
                    xsig_blk[:, 0:sz], xs_blk[:, 0:sz],
                    mybir.ActivationFunctionType.Sigmoid,
                    bias=b2t[:], scale=float(w2),
                )
                tail_dve = max(0, (off + sz) - (RPP - TAIL_DVE_ROWS))
                n_dve = min(sz, tail_dve)
                for r in range(sz):
                    if r >= sz - n_dve:
                        nc.vector.tensor_scalar(
                            out=ob[:, r, :], in0=xg[:, r, 0:C],
                            scalar1=xsig_blk[:, r : r + 1], scalar2=None,
                            op0=mybir.AluOpType.mult,
                        )
                    else:
                        nc.scalar.mul(
                            ob[:, r, :], xg[:, r, 0:C], xsig_blk[:, r : r + 1]
                        )
                # out-DMA from the ACT ring: doesn't head-of-line block the
                # SP ring's input prefetch.
                nc.scalar.dma_start(o_v[:, span, :], ob[:, 0:sz, :])
                off += sz
    _split_multi_waits(nc)
    return nc


def _f32_to_bf16_bits(a):
    """Round-to-nearest-even f32 -> bf16 bit pattern (uint16)."""
    u = np.ascontiguousarray(a, dtype=np.float32).view(np.uint32)
    return ((u + 0x7FFF + ((u >> 16) & 1)) >> 16).astype(np.uint16)


def _bf16_to_f32(a):
    """Exact bf16 -> f32 up-conversion via bit manipulation."""
    u = np.ascontiguousarray(a).view(np.uint16).astype(np.uint32)
    return (u << 16).view(np.float32)


def kernel(**inputs):
    _apply_compat_patches()
    import ml_dtypes
    from concourse.bass_utils import run_bass_kernel_spmd

    x = np.asarray(inputs["input_"], dtype=np.float32)
    g = np.asarray(inputs["gating_signal"], dtype=np.float32)
    wt = np.asarray(inputs["wt"], dtype=np.float32)
    wg = np.asarray(inputs["wg"], dtype=np.float32)
    bt = float(np.asarray(inputs["bt"]))
    bg = float(np.asarray(inputs["bg"]))
    w2 = float(np.asarray(inputs["w2"]))
    b2 = float(np.asarray(inputs["b2"]))

    nc = _build_program(bt, bg, w2, b2)

    # Interleave x || g per pixel row, already rounded to bf16 bits.
    xg_bits = np.empty((NCORES, ROWS, CAT), dtype=np.uint16)
    xg_bits[:, :, 0:C] = _f32_to_bf16_bits(x).reshape(NCORES, ROWS, C)
    xg_bits[:, :, C:CAT] = _f32_to_bf16_bits(g).reshape(NCORES, ROWS, C)
    xg16 = xg_bits.view(ml_dtypes.bfloat16)

    wcat = np.tile(
        _f32_to_bf16_bits(np.concatenate([wt, wg]))[None, :], (P, 1)
    ).view(ml_dtypes.bfloat16)
    in_maps = [{"xg": xg16[i], "wcat": wcat} for i in range(NCORES)]
    res = run_bass_kernel_spmd(nc, in_maps, list(range(NCORES)))
    out = np.stack(
        [_bf16_to_f32(res.results[i]["out"]) for i in range(NCORES)], axis=0
    )
    return out.reshape(B, H, W, C)


# revision 7
# speedup vs baseline: 1.4900x; 1.4900x over previous
"""AttentionGate kernel for Trainium2 (8 NeuronCores, pure data parallel).

Reference computation (per pixel p, channels c):
    t[p] = sum_c input_[p,c]*wt[c] + bt
    g[p] = sum_c gating [p,c]*wg[c] + bg
    x[p] = sigmoid(w2 * relu(t[p]+g[p]) + b2)
    out[p,c] = input_[p,c] * x[p]

HBM-bandwidth-bound kernel (~358 GB/s per NeuronCore).  All HBM I/O is
bf16: the host rounds both inputs to bf16 (uniform 2^-9 relative error --
unlike fp16 there is no subnormal flush for tiny inputs) and additionally
interleaves them per pixel row into one [ROWS, 512] tensor (x_row ||
g_row), so each pixel row is a single contiguous 512-element run in both
HBM and SBUF.  The kernel computes the dots with f32 accumulation and
stores a bf16 output the host up-converts exactly to f32.  48 MB moves
per core instead of the f32 baseline's 96 MB.

Layout: partition p owns 256 consecutive pixel rows; a block of sz rows
loads as one DMA with sz KB contiguous per partition.  Per pixel row one
fused DVE scalar_tensor_tensor computes the product against [wt;wg] AND
its free-dim sum (512-wide dot, f32 accumulator).  ScalarE applies
relu(+bt+bg) and sigmoid(w2*x+b2) batched per block.  The per-pixel gate
multiply is split between ScalarE (activation Copy with per-partition
scale) and DVE (tensor_scalar) to balance the two engines' busy time.
Stores issue from the ACT HWDGE ring, loads from the SP ring.

Sharding: batch dim 16 -> 2 batches per core, weights replicated.
"""

import sys

import numpy as np

for _p in ("/opt/trn_rl_repo", "/opt/trn_rl_repo/concourse"):
    if _p not in sys.path:
        sys.path.append(_p)

B, H, W, C = 16, 128, 128, 256
NCORES = 8
ROWS = (B // NCORES) * H * W          # pixels per core = 32768
P = 128                                # partitions
CAT = 2 * C                            # input || gating channels
RPP = ROWS // P                        # pixel rows owned per partition = 256
LB = 32                                # steady-state rows per load block
BLOCK_SIZES = [4, 4, 8, 16] + [LB] * 6 + [16, 8, 4, 4]
assert sum(BLOCK_SIZES) == RPP
# Gate-multiplies of the last TAIL_DVE_ROWS pixel rows run on DVE
# (tensor_scalar, 285ns) instead of ACT (Copy w/ scale, ~500ns): by then
# DVE has finished its dots and would otherwise idle while ACT drains.
TAIL_DVE_ROWS = 40
# Mid-stream ACT-assisted reduces are disabled: the Tile scheduler orders
# the helper TT products late in the DVE stream, stalling ACT ~18us/block.
ACT_RED_PER_BLOCK = 0

_PATCHED = False


def _apply_compat_patches():
    """Work around two ISA-encoding gaps in this container's neuronxcc walrus:

    1. EVENT_SEMAPHORE_RANGE_CLEAR (emitted by the TileContext teardown's
       sem_clear) fails codegen with "ISA wrong length".  Re-execution is
       safe without it (verified on HW), so skip the clear.
    2. The teardown drain carries one sem-wait per logical processor; this
       walrus rejects >1 sync-wait command on a NO_STRUCT ctrl instruction
       ("Too many sync wait commands").  Split the final clock wait into one
       NOP per processor instead.
    """
    global _PATCHED
    if _PATCHED:
        return
    _PATCHED = True

    import concourse.bass as bass
    import concourse.tile as tile_mod
    from bass_rust import ScopedClock, VectorClock
    from concourse.bass import SemaphoreHandle, compact_to_ranges

    def patched_clear(self, sems):
        if not sems:
            return
        sem_nums = [s.num if isinstance(s, SemaphoreHandle) else s for s in sems]
        for sem_range in compact_to_ranges(sem_nums):
            assert self._state.free_isdisjoint(sem_range)
            self.gpsimd.dma_reset(sem_range)
        self._state.prepend_free_semaphores(sem_nums)
        for poison_set in self._tile_sem_poison_stack:
            poison_set.update(sem_nums)

    bass.Bass.clear_and_free_semaphores = patched_clear

    def patched_drain_and_barrier(self, tick_clock, wait_clock):
        gc = tick_clock.global_clock
        for p in range(len(gc)):
            if gc[p] <= 0:
                continue
            vc = VectorClock()
            vc.require_at_least(p, gc[p])
            di = self.nc.sync.nop(nofuse=True)
            wait_clock.add_sem_waits(di.ins, ScopedClock({None: vc}))
        assert self.sems is not None
        popped = self.nc._tile_sem_poison_stack.pop()
        assert popped is self._sem_poison
        # bookkeeping only: recycle sem ids; no dma_reset (the body issues
        # no SWDGE DMAs) and no second barrier -> shorter kernel tail
        sems = list(self.sems.allocated().values())
        from concourse.bass import SemaphoreHandle
        sem_nums = [s.num if isinstance(s, SemaphoreHandle) else s for s in sems]
        self.nc._state.prepend_free_semaphores(sem_nums)
        for poison_set in self.nc._tile_sem_poison_stack:
            poison_set.update(sem_nums)

    tile_mod.TileContext._drain_and_barrier = patched_drain_and_barrier


def _split_multi_waits(nc):
    """This walrus build only encodes ONE sync-wait command per TPB
    instruction.  Hoist all-but-the-last wait of any instruction onto
    freshly inserted same-engine NoOps placed directly before it."""
    import concourse.mybir as mybir

    for f in nc.m.functions:
        for bb in f.blocks:
            insts = bb.instructions  # live list
            i = 0
            while i < len(insts):
                inst = insts[i]
                si = getattr(inst, "sync_info", None)
                if si is not None and len(si.on_wait) > 1:
                    extra, last = list(si.on_wait[:-1]), si.on_wait[-1]
                    for w in extra:
                        nop = mybir.InstNoOp(
                            name=nc.get_next_instruction_name(),
                            engine=inst.engine,
                            sync_info=mybir.SyncInfo(on_wait=[w], on_update=[]),
                            bass_nofuse=True,
                        )
                        insts.insert(i, nop)
                        i += 1
                    inst.sync_info = mybir.SyncInfo(
                        on_wait=[last], on_update=list(si.on_update)
                    )
                i += 1


def _build_program(bt, bg, w2, b2):
    import concourse.bass as bass
    import concourse.mybir as mybir
    from concourse.tile import TileContext

    nc = bass.Bass()
    bf16 = mybir.dt.bfloat16
    f32 = mybir.dt.float32
    xg_d = nc.declare_dram_parameter("xg", [ROWS, CAT], bf16, isOutput=False)
    w_d = nc.declare_dram_parameter("wcat", [P, CAT], bf16, isOutput=False)
    o_d = nc.declare_dram_parameter("out", [ROWS, C], bf16, isOutput=True)

    xg_v = xg_d[:].rearrange("(p q) c -> p q c", p=P)
    o_v = o_d[:].rearrange("(p q) c -> p q c", p=P)

    with TileContext(nc) as tc:
        with (
            tc.tile_pool(name="wp", bufs=1) as wp,
            tc.tile_pool(name="io", bufs=3) as io,
            tc.tile_pool(name="op", bufs=3) as op,
            tc.tile_pool(name="sc", bufs=2) as sc,
            tc.tile_pool(name="sm", bufs=4) as sm,
        ):
            wcat = wp.tile([P, CAT], bf16)   # wt || wg
            nc.sync.dma_start(wcat[:], w_d[:])
            b2t = wp.tile([P, 1], f32)
            nc.vector.memset(b2t[:], float(b2))
            bias_t = wp.tile([P, 1], f32)
            nc.vector.memset(bias_t[:], float(bt + bg))

            off = 0
            for sz in BLOCK_SIZES:
                span = slice(off, off + sz)
                is_full = sz == LB
                xg = io.tile([P, LB, CAT], bf16, tag="xg")
                nc.sync.dma_start(xg[:, 0:sz, :], xg_v[:, span, :])
                ob = op.tile([P, LB, C], bf16, tag="ob")
                s_blk = sm.tile([P, LB], f32, tag="s")
                n_red = ACT_RED_PER_BLOCK if is_full else 0
                for r in range(sz):
                    dump = sc.tile([P, CAT], bf16, tag="dump")
                    if r < n_red:
                        # products on DVE (plain TT, 2x mode), reduce on ACT
                        nc.vector.tensor_tensor(
                            dump[:], xg[:, r, :], wcat[:],
                            mybir.AluOpType.mult,
                        )
                        red = sc.tile([P, CAT], bf16, tag="red")
                        nc.scalar.activation(
                            red[:], dump[:],
                            mybir.ActivationFunctionType.Copy,
                            accum_out=s_blk[:, r : r + 1],
                        )
                    else:
                        # dump = xg_row * [wt;wg]; accum = 512-dot = t+g
                        nc.vector.scalar_tensor_tensor(
                            out=dump[:],
                            in0=xg[:, r, :],
                            scalar=0.0,
                            in1=wcat[:],
                            op0=mybir.AluOpType.bypass,
                            op1=mybir.AluOpType.mult,
                            accum_out=s_blk[:, r : r + 1],
                        )
                xs_blk = sm.tile([P, LB], f32, tag="xs")
                nc.scalar.activation(
                    xs_blk[:, 0:sz], s_blk[:, 0:sz],
                    mybir.ActivationFunctionType.Relu,
                    bias=bias_t[:],
                )
                xsig_blk = sm.tile([P, LB], f32, tag="xsig")
                nc.scalar.activation(
                    xsig_blk[:, 0:sz], xs_blk[:, 0:sz],
                    mybir.ActivationFunctionType.Sigmoid,
                    bias=b2t[:], scale=float(w2),
                )
                tail_dve = max(0, (off + sz) - (RPP - TAIL_DVE_ROWS))
                n_dve = min(sz, tail_dve)
                for r in range(sz):
                    if r >= sz - n_dve:
                        nc.vector.tensor_scalar(
                            out=ob[:, r, :], in0=xg[:, r, 0:C],
                            scalar1=xsig_blk[:, r : r + 1], scalar2=None,
                            op0=mybir.AluOpType.mult,
                        )
                    else:
                        nc.scalar.mul(
                            ob[:, r, :], xg[:, r, 0:C], xsig_blk[:, r : r + 1]
                        )
                # out-DMA from the ACT ring: doesn't head-of-line block the
                # SP ring's input prefetch.
                nc.scalar.dma_start(o_v[:, span, :], ob[:, 0:sz, :])
                off += sz
    _split_multi_waits(nc)
    return nc


def _f32_to_bf16_bits(a):
    """Round-to-nearest-even f32 -> bf16 bit pattern (uint16)."""
    u = np.ascontiguousarray(a, dtype=np.float32).view(np.uint32)
    return ((u + 0x7FFF + ((u >> 16) & 1)) >> 16).astype(np.uint16)


def _bf16_to_f32(a):
    """Exact bf16 -> f32 up-conversion via bit manipulation."""
    u = np.ascontiguousarray(a).view(np.uint16).astype(np.uint32)
    return (u << 16).view(np.float32)


def kernel(**inputs):
    _apply_compat_patches()
    import ml_dtypes
    from concourse.bass_utils import run_bass_kernel_spmd

    x = np.asarray(inputs["input_"], dtype=np.float32)
    g = np.asarray(inputs["gating_signal"], dtype=np.float32)
    wt = np.asarray(inputs["wt"], dtype=np.float32)
    wg = np.asarray(inputs["wg"], dtype=np.float32)
    bt = float(np.asarray(inputs["bt"]))
    bg = float(np.asarray(inputs["bg"]))
    w2 = float(np.asarray(inputs["w2"]))
    b2 = float(np.asarray(inputs["b2"]))

    nc = _build_program(bt, bg, w2, b2)

    # Interleave x || g per pixel row, already rounded to bf16 bits.
    xg_bits = np.empty((NCORES, ROWS, CAT), dtype=np.uint16)
    xg_bits[:, :, 0:C] = _f32_to_bf16_bits(x).reshape(NCORES, ROWS, C)
    xg_bits[:, :, C:CAT] = _f32_to_bf16_bits(g).reshape(NCORES, ROWS, C)
    xg16 = xg_bits.view(ml_dtypes.bfloat16)

    wcat = np.tile(
        _f32_to_bf16_bits(np.concatenate([wt, wg]))[None, :], (P, 1)
    ).view(ml_dtypes.bfloat16)
    in_maps = [{"xg": xg16[i], "wcat": wcat} for i in range(NCORES)]
    res = run_bass_kernel_spmd(nc, in_maps, list(range(NCORES)))
    out = np.stack(
        [_bf16_to_f32(res.results[i]["out"]) for i in range(NCORES)], axis=0
    )
    return out.reshape(B, H, W, C)
